# revision 8
# baseline (speedup 1.0000x reference)
"""Multi-head attention (B=4, S=2048, E=1024, H=16, D=64) on 8 TRN2 cores.

Sharding: heads 2c, 2c+1 on core c (Megatron-style column-parallel qkv,
row-parallel out-projection; partial outputs summed on host).

Per-core dataflow (all matmuls in float32r — full PE rate, ~1e-4 rel err):
  A) qkvT[384, 8192] = W_loc^T-style projection of the (replicated) xT,
     feature-major, heads packed [h0 | h1] on partitions per q/k/v tile.
  B) per (batch, head): scoresT[sk, sq] tiles via K=64 matmuls,
     exp on ScalarE (scale=1/8 folded in, no max-subtraction needed:
     |scores| < ~3), attnT accumulation with an appended ones-column on v
     producing softmax denominators as row 64 of the PSUM accumulator,
     then reciprocal + partition-broadcast-DMA + multiply to normalize.
  C) row-parallel out-projection of the local 128 features -> yT partial.
"""
import os
from contextlib import ExitStack

import numpy as np

import concourse.bass as bass
import concourse.mybir as mybir
import concourse.tile as tile
from concourse import bacc
from concourse.bass_utils import run_bass_kernel_spmd
from concourse.masks import make_identity

B, S, E, H, D = 4, 2048, 1024, 16, 64
NCORES = 8
HPC = H // NCORES        # 2 heads per core
F = HPC * D              # 128 local features
M3 = 3 * F               # 384 local qkv rows
BS = B * S               # 8192
KT_E = E // 128          # 8 contraction tiles for projections
KT_S = S // 128          # 16 sk tiles
f32 = mybir.dt.float32
f32r = mybir.dt.float32r
EXP = mybir.ActivationFunctionType.Exp

_prog_cache = {}


def build_program():
    if "nc" in _prog_cache:
        return _prog_cache["nc"]
    nc = bacc.Bacc("TRN2", target_bir_lowering=False)
    xT = nc.dram_tensor("xT", [E, BS], f32r, kind="ExternalInput")
    wq = nc.dram_tensor("wq", [E, M3], f32r, kind="ExternalInput")
    bq = nc.dram_tensor("bq", [128, 3], f32, kind="ExternalInput")
    wo = nc.dram_tensor("wo", [F, E], f32r, kind="ExternalInput")
    bo = nc.dram_tensor("bo", [128, E // 128], f32, kind="ExternalInput")
    yT = nc.dram_tensor("yT", [E, BS], f32, kind="ExternalOutput")

    with tile.TileContext(nc) as tc, ExitStack() as ctx:
        const = ctx.enter_context(tc.tile_pool(name="const", bufs=1))
        big = ctx.enter_context(tc.tile_pool(name="big", bufs=1))
        xp = ctx.enter_context(tc.tile_pool(name="xp", bufs=2))
        expp = ctx.enter_context(tc.tile_pool(name="expp", bufs=3))
        vkp = ctx.enter_context(tc.tile_pool(name="vkp", bufs=2 * KT_S))
        anp = ctx.enter_context(tc.tile_pool(name="anp", bufs=1))
        asp = ctx.enter_context(tc.tile_pool(name="asp", bufs=2))
        ystp = ctx.enter_context(tc.tile_pool(name="ystp", bufs=2))
        psmm = ctx.enter_context(tc.tile_pool(name="psmm", bufs=2, space="PSUM"))
        pssc = ctx.enter_context(tc.tile_pool(name="pssc", bufs=2, space="PSUM"))
        psat = ctx.enter_context(tc.tile_pool(name="psat", bufs=1, space="PSUM"))

        wq_sb = const.tile([128, KT_E, M3], f32r)
        nc.gpsimd.dma_start(out=wq_sb, in_=wq.rearrange("(kt p) m -> p kt m", p=128))
        wo_sb = const.tile([F, E], f32r)
        nc.gpsimd.dma_start(out=wo_sb, in_=wo[:, :])
        bq_sb = const.tile([128, 3], f32)
        nc.gpsimd.dma_start(out=bq_sb, in_=bq[:, :])
        bo_sb = const.tile([128, E // 128], f32)
        nc.gpsimd.dma_start(out=bo_sb, in_=bo[:, :])
        # identities for PE transposes of the two head slots (rows 0-63 / 64-127)
        id_f32 = const.tile([128, 64], f32)
        make_identity(nc, id_f32[0:64, :])
        make_identity(nc, id_f32[64:128, :])
        id_sb = const.tile([128, 64], f32r)
        nc.vector.tensor_copy(id_sb, id_f32)
        # f32r constant 1.0 column (walrus rejects f32r memset)
        ones_sb = const.tile([128, 1], f32r)
        nc.vector.tensor_scalar(
            ones_sb, wq_sb[:, 0, 0:1], 0.0, 1.0,
            mybir.AluOpType.mult, mybir.AluOpType.add)

        q_sb = big.tile([128, BS], f32r, tag="q")
        k_sb = big.tile([128, BS], f32r, tag="k")
        v_sb = big.tile([128, BS], f32r, tag="v")
        qkv_sb = [q_sb, k_sb, v_sb]
        xT_r = xT.rearrange("(kt p) n -> p kt n", p=128)

        # ---- Phase A: qkv projection, feature-major ----
        for n in range(BS // 512):
            xc = xp.tile([128, KT_E, 512], f32r, tag="xc")
            nc.sync.dma_start(out=xc, in_=xT_r[:, :, n * 512:(n + 1) * 512])
            for m in range(3):
                ps = psmm.tile([128, 512], f32, tag="mm")
                for kt in range(KT_E):
                    nc.tensor.matmul(
                        ps,
                        lhsT=wq_sb[:, kt, m * 128:(m + 1) * 128],
                        rhs=xc[:, kt, :],
                        start=(kt == 0), stop=(kt == KT_E - 1))
                nc.vector.tensor_scalar_add(
                    qkv_sb[m][:, n * 512:(n + 1) * 512], ps, bq_sb[:, m:m + 1])

        # ---- Phase B+C per batch ----
        for b in range(B):
            ab = asp.tile([128, S], f32r, tag="ab")
            # v transposes: vT [64, S] -> per-sk-tile [128, 64] + ones column
            vk = {}
            for h in range(HPC):
                hb = h * 64
                for kt in range(KT_S):
                    vt = psmm.tile([128, 64], f32r, tag="mm")
                    nc.tensor.transpose(
                        vt,
                        in_=v_sb[hb:hb + 64, b * S + kt * 128:b * S + (kt + 1) * 128],
                        identity=id_sb[hb:hb + 64, :])
                    vkt = vkp.tile([128, 65], f32r, tag="vk")
                    nc.vector.tensor_copy(vkt[:, 0:64], vt)
                    nc.vector.tensor_copy(vkt[:, 64:65], ones_sb)
                    vk[(h, kt)] = vkt

            for h in range(HPC):
                hb = h * 64
                for c in range(2):  # sq chunks of 1024
                    cq = b * S + c * 1024
                    at = psat.tile([65, 1024], f32, tag="at")
                    for kt in range(KT_S):
                        sc = pssc.tile([128, 1024], f32, tag="sc")
                        for u in range(2):
                            nc.tensor.matmul(
                                sc[:, u * 512:(u + 1) * 512],
                                lhsT=k_sb[hb:hb + 64,
                                          b * S + kt * 128:b * S + (kt + 1) * 128],
                                rhs=q_sb[hb:hb + 64, cq + u * 512:cq + (u + 1) * 512],
                                start=True, stop=True)
                        ex = expp.tile([128, 1024], f32r, tag="exp")
                        nc.scalar.activation(ex, sc, EXP, scale=0.125)
                        for u in range(2):
                            nc.tensor.matmul(
                                at[:, u * 512:(u + 1) * 512],
                                lhsT=vk[(h, kt)],
                                rhs=ex[:, u * 512:(u + 1) * 512],
                                start=(kt == 0), stop=(kt == KT_S - 1))
                    # normalize: recip of sums row, broadcast to 64 partitions
                    # recip lands at partition 64; partition_broadcast on HW
                    # reads partition 0 only, so DMA-bounce the row down.
                    rs = anp.tile([65, 1024], f32, tag="rs")
                    nc.vector.reciprocal(rs[64:65, :], at[64:65, :])
                    nc.sync.dma_start(out=rs[0:1, :], in_=rs[64:65, :])
                    rb = anp.tile([64, 1024], f32, tag="rb")
                    nc.gpsimd.partition_broadcast(rb, rs[0:1, :])
                    if h == 0:
                        nc.vector.tensor_mul(
                            ab[0:64, c * 1024:(c + 1) * 1024],
                            at[0:64, :], rb)
                    else:
                        nm = anp.tile([64, 1024], f32r, tag="nm")
                        nc.vector.tensor_mul(nm, at[0:64, :], rb)
                        nc.sync.dma_start(
                            out=ab[64:128, c * 1024:(c + 1) * 1024], in_=nm)

            # ---- Phase C: out-projection for batch b ----
            for o in range(E // 128):
                for c4 in range(4):
                    yp = psmm.tile([128, 512], f32, tag="mm")
                    nc.tensor.matmul(
                        yp,
                        lhsT=wo_sb[:, o * 128:(o + 1) * 128],
                        rhs=ab[:, c4 * 512:(c4 + 1) * 512],
                        start=True, stop=True)
                    yst = ystp.tile([128, 512], f32, tag="yst")
                    nc.vector.tensor_scalar_add(yst, yp, bo_sb[:, o:o + 1])
                    nc.gpsimd.dma_start(
                        out=yT[o * 128:(o + 1) * 128,
                               b * S + c4 * 512:b * S + (c4 + 1) * 512],
                        in_=yst)

    nc.compile()
    _prog_cache["nc"] = nc
    return nc


def make_in_maps(x, W_qkv, b_qkv, W_out, b_out):
    xT = np.ascontiguousarray(x.reshape(BS, E).T).astype(np.float32)
    in_maps = []
    for c in range(NCORES):
        rows, brows = [], []
        for blk in range(3):
            for h in (HPC * c, HPC * c + 1):
                rows.append(W_qkv[blk * E + h * D: blk * E + (h + 1) * D, :])
                brows.append(b_qkv[blk * E + h * D: blk * E + (h + 1) * D])
        W_loc = np.concatenate(rows, axis=0)            # [384, 1024]
        b_loc = np.concatenate(brows, axis=0)           # [384]
        wq_in = np.ascontiguousarray(W_loc.T).astype(np.float32)
        bq_in = np.ascontiguousarray(b_loc.reshape(3, 128).T).astype(np.float32)
        wo_in = np.ascontiguousarray(
            W_out[:, c * F:(c + 1) * F].T).astype(np.float32)
        if c == 0:
            bo_in = np.ascontiguousarray(
                b_out.reshape(E // 128, 128).T).astype(np.float32)
        else:
            bo_in = np.zeros((128, E // 128), dtype=np.float32)
        in_maps.append(
            {"xT": xT, "wq": wq_in, "bq": bq_in, "wo": wo_in, "bo": bo_in})
    return in_maps


def kernel(x, W_qkv, b_qkv, W_out, b_out):
    x = np.asarray(x, dtype=np.float32)
    W_qkv = np.asarray(W_qkv, dtype=np.float32)
    b_qkv = np.asarray(b_qkv, dtype=np.float32)
    W_out = np.asarray(W_out, dtype=np.float32)
    b_out = np.asarray(b_out, dtype=np.float32)

    nc = build_program()
    in_maps = make_in_maps(x, W_qkv, b_qkv, W_out, b_out)
    res = run_bass_kernel_spmd(nc, in_maps, core_ids=list(range(NCORES)))
    acc = np.zeros((E, BS), dtype=np.float32)
    for c in range(NCORES):
        acc += res.results[c]["yT"]
    return np.ascontiguousarray(acc.T).reshape(B, S, E)


if __name__ == "__main__":
    rng = np.random.default_rng(0)
    x = rng.standard_normal((B, S, E), dtype=np.float32)
    s = 1.0 / np.sqrt(E)
    W_qkv = rng.uniform(-s, s, (3 * E, E)).astype(np.float32)
    b_qkv = rng.uniform(-s, s, (3 * E,)).astype(np.float32)
    W_out = rng.uniform(-s, s, (E, E)).astype(np.float32)
    b_out = rng.uniform(-s, s, (E,)).astype(np.float32)
    y = kernel(x, W_qkv, b_qkv, W_out, b_out)
    print("out", y.shape, y.dtype, float(np.abs(y).max()))


# revision 10
# speedup vs baseline: 1.4694x; 1.4694x over previous
"""Multi-head attention (B=4, S=2048, E=1024, H=16, D=64) on 8 TRN2 cores.

Sharding: heads 2c, 2c+1 on core c (Megatron-style column-parallel qkv,
row-parallel out-projection; partial outputs summed on host).

Per-core dataflow (all matmuls in float32r — full PE rate, ~1e-4 rel err):
  A) qkvT[384, 8192] feature-major projection of the (replicated) xT,
     heads packed [h0 | h1] on partitions per q/k/v tile.
  B) per (batch, head, sq-chunk): scoresT[sk, sq] via K=64 matmuls, exp on
     ScalarE (scale=1/8 folded in; no max-subtraction needed, |scores|<3),
     attnT accumulation with an appended ones-column on v producing softmax
     denominators as PSUM row 64, then reciprocal + partition-broadcast +
     multiply to normalize.
  C) row-parallel out-projection of the local 128 features -> yT partial.

Emission interleaves phase-A chunks of batch b+1 and out-projection parts of
batch b-1 between attention chunk groups of batch b, so the in-order PE
always has dependency-free matmuls to fill ACT/normalization stalls.
"""
from contextlib import ExitStack

import numpy as np

import concourse.bass as bass
import concourse.mybir as mybir
import concourse.tile as tile
from concourse import bacc
from concourse.bass_utils import run_bass_kernel_spmd
from concourse.masks import make_identity

B, S, E, H, D = 4, 2048, 1024, 16, 64
NCORES = 8
HPC = H // NCORES        # 2 heads per core
F = HPC * D              # 128 local features
M3 = 3 * F               # 384 local qkv rows
BS = B * S               # 8192
KT_E = E // 128          # 8 contraction tiles for projections
KT_S = S // 128          # 16 sk tiles
f32 = mybir.dt.float32
f32r = mybir.dt.float32r
EXP = mybir.ActivationFunctionType.Exp

_prog_cache = {}


def build_program():
    if "nc" in _prog_cache:
        return _prog_cache["nc"]
    nc = bacc.Bacc("TRN2", target_bir_lowering=False)
    xT = nc.dram_tensor("xT", [E, BS], f32r, kind="ExternalInput")
    wq = nc.dram_tensor("wq", [E, M3], f32r, kind="ExternalInput")
    bq = nc.dram_tensor("bq", [128, 3], f32, kind="ExternalInput")
    wo = nc.dram_tensor("wo", [F, E], f32r, kind="ExternalInput")
    bo = nc.dram_tensor("bo", [128, E // 128], f32, kind="ExternalInput")
    yT = nc.dram_tensor("yT", [E, BS], f32, kind="ExternalOutput")

    with tile.TileContext(nc) as tc, ExitStack() as ctx:
        const = ctx.enter_context(tc.tile_pool(name="const", bufs=1))
        big = ctx.enter_context(tc.tile_pool(name="big", bufs=1))
        xp = ctx.enter_context(tc.tile_pool(name="xp", bufs=2))
        expp = ctx.enter_context(tc.tile_pool(name="expp", bufs=2))
        vkp = ctx.enter_context(tc.tile_pool(name="vkp", bufs=2 * KT_S))
        anp = ctx.enter_context(tc.tile_pool(name="anp", bufs=4))
        asp = ctx.enter_context(tc.tile_pool(name="asp", bufs=2))
        ystp = ctx.enter_context(tc.tile_pool(name="ystp", bufs=4))
        psmm = ctx.enter_context(tc.tile_pool(name="psmm", bufs=2, space="PSUM"))
        pssc = ctx.enter_context(tc.tile_pool(name="pssc", bufs=2, space="PSUM"))
        psat = ctx.enter_context(tc.tile_pool(name="psat", bufs=1, space="PSUM"))

        wq_sb = const.tile([128, KT_E, M3], f32r)
        nc.gpsimd.dma_start(out=wq_sb, in_=wq.rearrange("(kt p) m -> p kt m", p=128))
        wo_sb = const.tile([F, E], f32r)
        nc.gpsimd.dma_start(out=wo_sb, in_=wo[:, :])
        bq_sb = const.tile([128, 3], f32)
        nc.gpsimd.dma_start(out=bq_sb, in_=bq[:, :])
        bo_sb = const.tile([128, E // 128], f32)
        nc.gpsimd.dma_start(out=bo_sb, in_=bo[:, :])
        # identities for PE transposes of the two head slots (rows 0-63 / 64-127)
        id_f32 = const.tile([128, 64], f32)
        make_identity(nc, id_f32[0:64, :])
        make_identity(nc, id_f32[64:128, :])
        id_sb = const.tile([128, 64], f32r)
        nc.vector.tensor_copy(id_sb, id_f32)
        # f32r constant 1.0 column (walrus rejects f32r memset)
        ones_sb = const.tile([128, 1], f32r)
        nc.vector.tensor_scalar(
            ones_sb, wq_sb[:, 0, 0:1], 0.0, 1.0,
            mybir.AluOpType.mult, mybir.AluOpType.add)

        q_sb = big.tile([128, BS], f32r, tag="q")
        k_sb = big.tile([128, BS], f32r, tag="k")
        v_sb = big.tile([128, BS], f32r, tag="v")
        qkv_sb = [q_sb, k_sb, v_sb]
        xT_r = xT.rearrange("(kt p) n -> p kt n", p=128)

        def emit_A_chunk(n):
            """qkv projection for columns [n*512, (n+1)*512)."""
            xc = xp.tile([128, KT_E, 512], f32r, tag="xc")
            nc.sync.dma_start(out=xc, in_=xT_r[:, :, n * 512:(n + 1) * 512])
            for m in range(3):
                ps = psmm.tile([128, 512], f32, tag="mm")
                for kt in range(KT_E):
                    nc.tensor.matmul(
                        ps,
                        lhsT=wq_sb[:, kt, m * 128:(m + 1) * 128],
                        rhs=xc[:, kt, :],
                        start=(kt == 0), stop=(kt == KT_E - 1))
                nc.vector.tensor_scalar_add(
                    qkv_sb[m][:, n * 512:(n + 1) * 512], ps, bq_sb[:, m:m + 1])

        def emit_vt(b, h, kt, vk):
            hb = h * 64
            vt = psmm.tile([128, 64], f32r, tag="mm")
            nc.tensor.transpose(
                vt,
                in_=v_sb[hb:hb + 64, b * S + kt * 128:b * S + (kt + 1) * 128],
                identity=id_sb[hb:hb + 64, :])
            vkt = vkp.tile([128, 65], f32r, tag="vk")
            nc.vector.tensor_copy(vkt[:, 0:64], vt)
            nc.vector.tensor_copy(vkt[:, 64:65], ones_sb)
            vk[(h, kt)] = vkt

        def emit_attn_group(b, h, c, vk, ab):
            hb = h * 64
            cq = b * S + c * 1024
            at = psat.tile([65, 1024], f32, tag="at")
            for kt in range(KT_S):
                sc = pssc.tile([128, 1024], f32, tag="sc")
                for u in range(2):
                    nc.tensor.matmul(
                        sc[:, u * 512:(u + 1) * 512],
                        lhsT=k_sb[hb:hb + 64,
                                  b * S + kt * 128:b * S + (kt + 1) * 128],
                        rhs=q_sb[hb:hb + 64, cq + u * 512:cq + (u + 1) * 512],
                        start=True, stop=True)
                ex = expp.tile([128, 1024], f32r, tag="exp")
                nc.scalar.activation(ex, sc, EXP, scale=0.125)
                for u in range(2):
                    nc.tensor.matmul(
                        at[:, u * 512:(u + 1) * 512],
                        lhsT=vk[(h, kt)],
                        rhs=ex[:, u * 512:(u + 1) * 512],
                        start=(kt == 0), stop=(kt == KT_S - 1))
            # normalize: recip of PSUM row 64 (softmax denominator), bounce
            # to partition 0 (HW partition_broadcast reads partition 0 only),
            # broadcast, multiply.
            rs = anp.tile([65, 1024], f32, tag="norm")
            nc.vector.reciprocal(rs[64:65, :], at[64:65, :])
            nc.sync.dma_start(out=rs[0:1, :], in_=rs[64:65, :])
            rb = anp.tile([64, 1024], f32, tag="norm")
            nc.gpsimd.partition_broadcast(rb, rs[0:1, :])
            if h == 0:
                nc.vector.tensor_mul(
                    ab[0:64, c * 1024:(c + 1) * 1024], at[0:64, :], rb)
            else:
                nm = anp.tile([64, 1024], f32r, tag="norm")
                nc.vector.tensor_mul(nm, at[0:64, :], rb)
                nc.sync.dma_start(
                    out=ab[64:128, c * 1024:(c + 1) * 1024], in_=nm)

        def emit_outproj_part(b, part, ab):
            """2 of the 8 output o-tiles for batch b."""
            for o in (2 * part, 2 * part + 1):
                for c4 in range(4):
                    yp = psmm.tile([128, 512], f32, tag="mm")
                    nc.tensor.matmul(
                        yp,
                        lhsT=wo_sb[:, o * 128:(o + 1) * 128],
                        rhs=ab[:, c4 * 512:(c4 + 1) * 512],
                        start=True, stop=True)
                    yst = ystp.tile([128, 512], f32, tag="yst")
                    nc.vector.tensor_scalar_add(yst, yp, bo_sb[:, o:o + 1])
                    eng = nc.sync if (o + c4) % 2 else nc.gpsimd
                    eng.dma_start(
                        out=yT[o * 128:(o + 1) * 128,
                               b * S + c4 * 512:b * S + (c4 + 1) * 512],
                        in_=yst)

        # ---- interleaved emission schedule ----
        for n in range(4):           # phase A for batch 0
            emit_A_chunk(n)
        abs_ = {}
        for b in range(B):
            abs_[b] = asp.tile([128, S], f32r, tag="ab", name=f"ab{b}")
            vk = {}
            for h in range(HPC):
                for kt in range(KT_S):
                    emit_vt(b, h, kt, vk)
            for gi, (h, c) in enumerate([(0, 0), (0, 1), (1, 0), (1, 1)]):
                emit_attn_group(b, h, c, vk, abs_[b])
                if b + 1 < B:
                    emit_A_chunk(4 * (b + 1) + gi)
                if b >= 1:
                    emit_outproj_part(b - 1, gi, abs_[b - 1])
        for gi in range(4):
            emit_outproj_part(B - 1, gi, abs_[B - 1])

    nc.compile()
    _prog_cache["nc"] = nc
    return nc


def make_in_maps(x, W_qkv, b_qkv, W_out, b_out):
    xT = np.ascontiguousarray(x.reshape(BS, E).T).astype(np.float32)
    in_maps = []
    for c in range(NCORES):
        rows, brows = [], []
        for blk in range(3):
            for h in (HPC * c, HPC * c + 1):
                rows.append(W_qkv[blk * E + h * D: blk * E + (h + 1) * D, :])
                brows.append(b_qkv[blk * E + h * D: blk * E + (h + 1) * D])
        W_loc = np.concatenate(rows, axis=0)            # [384, 1024]
        b_loc = np.concatenate(brows, axis=0)           # [384]
        wq_in = np.ascontiguousarray(W_loc.T).astype(np.float32)
        bq_in = np.ascontiguousarray(b_loc.reshape(3, 128).T).astype(np.float32)
        wo_in = np.ascontiguousarray(
            W_out[:, c * F:(c + 1) * F].T).astype(np.float32)
        if c == 0:
            bo_in = np.ascontiguousarray(
                b_out.reshape(E // 128, 128).T).astype(np.float32)
        else:
            bo_in = np.zeros((128, E // 128), dtype=np.float32)
        in_maps.append(
            {"xT": xT, "wq": wq_in, "bq": bq_in, "wo": wo_in, "bo": bo_in})
    return in_maps


def kernel(x, W_qkv, b_qkv, W_out, b_out):
    x = np.asarray(x, dtype=np.float32)
    W_qkv = np.asarray(W_qkv, dtype=np.float32)
    b_qkv = np.asarray(b_qkv, dtype=np.float32)
    W_out = np.asarray(W_out, dtype=np.float32)
    b_out = np.asarray(b_out, dtype=np.float32)

    nc = build_program()
    in_maps = make_in_maps(x, W_qkv, b_qkv, W_out, b_out)
    res = run_bass_kernel_spmd(nc, in_maps, core_ids=list(range(NCORES)))
    acc = np.zeros((E, BS), dtype=np.float32)
    for c in range(NCORES):
        acc += res.results[c]["yT"]
    return np.ascontiguousarray(acc.T).reshape(B, S, E)


if __name__ == "__main__":
    rng = np.random.default_rng(0)
    x = rng.standard_normal((B, S, E), dtype=np.float32)
    s = 1.0 / np.sqrt(E)
    W_qkv = rng.uniform(-s, s, (3 * E, E)).astype(np.float32)
    b_qkv = rng.uniform(-s, s, (3 * E,)).astype(np.float32)
    W_out = rng.uniform(-s, s, (E, E)).astype(np.float32)
    b_out = rng.uniform(-s, s, (E,)).astype(np.float32)
    y = kernel(x, W_qkv, b_qkv, W_out, b_out)
    print("out", y.shape, y.dtype, float(np.abs(y).max()))
